# revision 13
# baseline (speedup 1.0000x reference)
"""Trainium2 Bass kernel for CCPLoss:
out = sigmoid(mean(|maxpool35(min_c restored) - maxpool35(min_c target)|))

Inputs: restored, target: [16, 3, 512, 512] fp32.
Sharding: pure data parallel over batch; 2 images per core on 8 cores.

Algorithm: log-sum-exp pooling on the Tensor engine. maxpool35 is computed
as (1/B)*ln(sum_{window} exp(B*(x-1))), B=192, which is exact to ~ln(neff)/B
(~3e-4 here) and whose residual bias cancels between the two images inside
the |r-t| mean. Both windowed-sum axes become banded matmuls on the (otherwise
idle) PE array, each fused with a transpose by using the image as the
stationary operand and a constant 35-wide band matrix as the moving operand:

  stage1: F[w, i_h] = sum_h E[h, w] * Band[h, i_h]   (H-axis sum + transpose)
  stage2: G[i_h, i_w] = sum_w F[w, i_h] * Band[w, i_w] (W-axis sum + transpose)

Engine budget per core-iteration: DMA ~19us (bf16 inputs, host pre-cast),
DVE ~11us (channel-min + diff), ACT ~18us (exp, ln-from-PSUM, abs+accum),
PE ~20us (banded matmuls), Pool ~12us (PSUM->SBUF copies).
Host: sum partials in float64, mean /B, sigmoid.
"""

import sys

for _p in ("/opt/trn_rl_repo",):
    if _p not in sys.path:
        sys.path.insert(0, _p)

import numpy as np
import ml_dtypes

import concourse.bass as bass
import concourse.mybir as mybir
from concourse import bacc
from concourse.tile import TileContext

F32 = mybir.dt.float32
F16 = mybir.dt.float16
BF16 = mybir.dt.bfloat16
ALU = mybir.AluOpType
AFT = mybir.ActivationFunctionType

N_CORES = 8
B_FULL = 16
B_PER_CORE = B_FULL // N_CORES  # 2
C = 3
H = W = 512
K = 35
PAD = K // 2  # 17
NCH = 4  # 128-row chunks per image
FD = NCH * W  # 2048 columns per image
BW = 128 + K - 1  # 162: band tile width
BETA = 192.0

_COMPILED = None


def _band_np():
    """Band[p, k] = 1 iff p <= k <= p+34 (bf16)."""
    p = np.arange(128)[:, None]
    k = np.arange(BW)[None, :]
    return ((k >= p) & (k <= p + K - 1)).astype(ml_dtypes.bfloat16)


# Per contributing input chunk c: (fresh out-col range, fresh band k0,
# acc out-col range, acc band k0). Out col i of chunk c uses band col
# k = i - 128*c + PAD. Fresh regions partition [0,512); acc regions add the
# boundary overlap into the previous fresh region.
def _mm_plan():
    plan = []
    for c in range(NCH):
        lo = 128 * c - PAD  # first out col receiving chunk c
        hi = min(512, 128 * c + 128 + PAD)  # one past last
        if c == 0:
            fresh = (0, 145)  # k in [17, 162)
            acc = None
        else:
            fresh = (128 * c + PAD, hi)  # k in [34, ...)
            acc = (lo, 128 * c + PAD)  # k in [0, 34)
        plan.append((c, fresh, acc))
    return plan


_PLAN = _mm_plan()


def _build_nc(reps=1, loops=1):
    """reps: python-unrolled iterations of the workload. loops: hardware
    For_i loop around the unrolled body (timing only; multiplies reps)."""
    nc = bacc.Bacc("TRN2", detect_race_conditions=False)
    restored = nc.declare_dram_parameter(
        "restored", [B_PER_CORE, C, H, W], BF16, isOutput=False
    )
    target = nc.declare_dram_parameter(
        "target", [B_PER_CORE, C, H, W], BF16, isOutput=False
    )
    partial = nc.declare_dram_parameter("partial", [128, 1], F32, isOutput=True)
    band_dram = nc.inline_tensor(np.asarray(_band_np()), name="band_const")

    with (
        TileContext(nc) as tc,
        tc.tile_pool(name="const", bufs=1) as cpool,
        tc.tile_pool(name="work", bufs=1) as pool,
        tc.tile_pool(name="psum", bufs=1, space="PSUM") as ppool,
    ):
        band = cpool.tile([128, BW], BF16)
        nc.sync.dma_start(band[:], band_dram[:])
        smax = cpool.tile([128, 1], F32)
        nc.vector.memset(smax[:], 0.0)
        sc_beta = cpool.tile([128, 1], F32)
        nc.vector.memset(sc_beta[:], BETA)
        b_nbeta = cpool.tile([128, 1], F32)
        nc.vector.memset(b_nbeta[:], -BETA)
        b_eps = cpool.tile([128, 1], F32)
        nc.vector.memset(b_eps[:], 1e-30)

        def load_pair(b, rep):
            """Both images of pair b, channel-major: Xc[p, ch, img, c, w]."""
            Xc = pool.tile([128, C * 2 * FD], BF16, tag="Xc", bufs=2,
                           name=f"Xc_{b}_{rep}")
            Xc5 = Xc.rearrange("p (ch i c w) -> p ch i c w", ch=C, i=2, w=W)
            for ch in range(C):
                for i, inp in enumerate((restored, target)):
                    src = inp[b, ch].rearrange("(c p) w -> p c w", p=128)
                    nc.sync.dma_start(Xc5[:, ch, i], src)
            return Xc

        def banded_mms(out_ps, out_off, weights_of_c, who):
            """7 matmuls accumulating sum_{v in window(i)} X[v, j] into
            out_ps[:, out_off + i] for i in [0, 512). weights_of_c(c) gives
            the [128, 128] stationary slice for input chunk c."""
            for c, fresh, acc in _PLAN:
                wt = weights_of_c(c)
                if acc is not None:
                    a0, a1 = acc
                    nc.tensor.matmul(
                        out_ps[:, out_off + a0 : out_off + a1],
                        wt,
                        band[:, 0 : a1 - a0],
                        start=False, stop=True, skip_group_check=True,
                    )
                f0, f1 = fresh
                k0 = (f0 - 128 * c + PAD)
                nc.tensor.matmul(
                    out_ps[:, out_off + f0 : out_off + f1],
                    wt,
                    band[:, k0 : k0 + (f1 - f0)],
                    start=True, stop=True, skip_group_check=True,
                )

        def pool_image(E5, b, i, rep):
            """LSE-pool one image (E5[p, c, w] = exp tile view for image i).
            Returns U [128, FD] f16 = ln(G) in natural chunk layout."""
            who = f"{b}_{i}_{rep}"
            Fs = []
            for ws in range(4):
                Fp = ppool.tile([128, 512], F32, tag="F", bufs=3,
                                name=f"F_{who}_{ws}")
                banded_mms(
                    Fp, 0,
                    lambda c: E5[:, c, ws * 128 : (ws + 1) * 128],
                    f"s1_{who}_{ws}",
                )
                Fsb = pool.tile([128, 512], BF16, tag="Fs", bufs=8,
                                name=f"Fs_{who}_{ws}")
                nc.vector.tensor_copy(Fsb[:], Fp[:])
                Fs.append(Fsb)
            U = pool.tile([128, FD], F16, tag="U", bufs=4, name=f"U_{who}")
            Gp = ppool.tile([128, FD], F32, tag="G", bufs=1, name=f"G_{who}")
            for m in range(4):
                banded_mms(
                    Gp, m * W,
                    lambda s: Fs[s][:, m * 128 : (m + 1) * 128],
                    f"s2_{who}_{m}",
                )
            nc.scalar.activation(U[:], Gp[:], AFT.Ln, bias=b_eps[:])
            return U

        import contextlib

        def body(rep):
            for b in range(B_PER_CORE):
                Xc = load_pair(b, rep)
                Xc5 = Xc.rearrange("p (ch i c w) -> p ch i c w", ch=C, i=2, w=W)
                Xc3 = Xc.rearrange("p (ch m) -> p ch m", ch=C)
                M = pool.tile([128, 2 * FD], BF16, tag="M", bufs=2,
                              name=f"M_{b}_{rep}")
                nc.vector.tensor_tensor(M[:], Xc3[:, 1], Xc3[:, 2], ALU.min)
                nc.vector.tensor_tensor(M[:], M[:], Xc3[:, 0], ALU.min)
                E = pool.tile([128, 2 * FD], BF16, tag="E", bufs=2,
                              name=f"E_{b}_{rep}")
                nc.scalar.activation(
                    E[:], M[:], AFT.Exp, scale=sc_beta[:], bias=b_nbeta[:]
                )
                E6 = E.rearrange("p (i c w) -> p i c w", i=2, w=W)
                Ur = pool_image(E6[:, 0], b, 0, rep)
                Ut = pool_image(E6[:, 1], b, 1, rep)
                D = pool.tile([128, FD], F16, tag="D", bufs=2, name=f"D_{b}_{rep}")
                nc.gpsimd.tensor_tensor(D[:], Ur[:], Ut[:], ALU.subtract)
                Ab = pool.tile([128, FD], F16, tag="Ab", bufs=2,
                               name=f"Ab_{b}_{rep}")
                amax = pool.tile([128, 1], F32, tag="amax", bufs=4,
                                 name=f"am_{b}_{rep}")
                nc.scalar.activation(Ab[:], D[:], AFT.Abs, accum_out=amax[:])
                nc.vector.tensor_tensor(smax[:], smax[:], amax[:], ALU.add)

        loop_ctx = tc.For_i(0, loops) if loops > 1 else contextlib.nullcontext()
        with loop_ctx:
            for rep in range(reps):
                body(rep)
        out1 = pool.tile([128, 1], F32)
        nc.vector.tensor_copy(out1[:], smax[:])
        nc.sync.dma_start(partial[:], out1[:])

    nc.compile()
    return nc


def _get_compiled():
    global _COMPILED
    if _COMPILED is None:
        _COMPILED = _build_nc()
    return _COMPILED


def _to_bf16(x):
    return np.ascontiguousarray(x, dtype=np.float32).astype(ml_dtypes.bfloat16)


def kernel(restored: np.ndarray, target: np.ndarray) -> np.ndarray:
    from concourse.bass_utils import run_bass_kernel_spmd

    restored = _to_bf16(restored)
    target = _to_bf16(target)
    nc = _get_compiled()
    in_maps = []
    for i in range(N_CORES):
        sl = slice(i * B_PER_CORE, (i + 1) * B_PER_CORE)
        in_maps.append(
            {
                "restored": np.ascontiguousarray(restored[sl]),
                "target": np.ascontiguousarray(target[sl]),
            }
        )
    res = run_bass_kernel_spmd(nc, in_maps, list(range(N_CORES)))
    total = np.float64(0.0)
    for r in res.results:
        p = np.asarray(r["partial"], dtype=np.float64)
        total += p[:, 0].sum()
    mean = total / (BETA * float(B_FULL * H * W))
    out = 1.0 / (1.0 + np.exp(-mean))
    return np.asarray(out, dtype=np.float32)
